# revision 14
# baseline (speedup 1.0000x reference)
"""Trainium2 Bass kernel for nn_AdaptiveAttention (8-core data parallel).

Sharding strategy (host side, inside kernel()):
  - batch dim (256) sharded over 8 cores -> 32 batches/core.
  - all weights replicated (passed bf16; compute is bf16 on the PE).
  - spatial_image shard is passed in BOTH layouts as bf16:
      xt = X^T [1024, 6272]  (features on partitions; rhs of the big matmul)
      xn = X   [6272, 1024]  (rows on partitions; rhs of the context matmul)
    Two bf16 copies == same HBM bytes as one f32 copy.
  - small per-batch tensors / biases are passed pre-transposed / pre-tiled so
    every on-chip bias is per-partition and no on-chip transposes of inputs
    are needed.

On-chip math (per core, "transposed world": features on partitions, batch on
free dim):
  saT  = relu(Wsa^T @ st^T + bsa)            [1024, 32]
  haT  = tanh(Wha^T @ dec^T + bha)           [1024, 32]
  hatT = Wht^T @ haT                          [512, 32]
  senT = tanh(Wst^T @ saT + hatT + bst+bht)   [512, 32]
  alpha_sen[b] = walpha . senT[:, b]          via PE (lhsT=senT chunk)
  V^T  = Wva^T @ X^T (+ hatT + bvt+bht bcast) -> tanh -> alpha = walpha . (.)
  softmax over [32, 197] (b on partitions) -> att_weights (output)
  context = Wblk^T @ X + e_sen * sa           (Wblk = block-diag of UNNORMALIZED
                                               exp weights: mask * exp; the
                                               1/sum normalization is folded
                                               into the context epilogue)
  out_l^T = tanh(Wctx^T @ (context^T + haT) + bctx)   [1024, 32]
"""

import numpy as np
import ml_dtypes

import concourse.bass as bass
import concourse.bacc as bacc
import concourse.tile as tile
import concourse.mybir as mybir
from concourse.bass_utils import run_bass_kernel_spmd

BF16 = mybir.dt.bfloat16
F32 = mybir.dt.float32
AF = mybir.ActivationFunctionType
ALU = mybir.AluOpType
AX = mybir.AxisListType

NCORES = 8
B, P, H, A = 256, 196, 1024, 512
NB = B // NCORES            # 32 batches per core
R = NB * P                  # 6272 rows per core
KH = H // 128               # 8 h-chunks
KA = A // 128               # 4 a-chunks
GB = 2                      # batches per V-group
NG = NB // GB               # 16 groups
GR = GB * P                 # 392 rows per group
RC = R // 128               # 49 row-chunks for context
XNC = 7                     # xn tiles consolidated per DMA

_BF = ml_dtypes.bfloat16


def build_nc():
    nc = bacc.Bacc(None, target_bir_lowering=False, debug=False)

    xt = nc.declare_dram_parameter("xt", [H, R], BF16, isOutput=False)
    xn = nc.declare_dram_parameter("xn", [R, H], BF16, isOutput=False)
    dot_t = nc.declare_dram_parameter("dot_t", [H, NB], BF16, isOutput=False)
    st_t = nc.declare_dram_parameter("st_t", [H, NB], BF16, isOutput=False)
    wsa = nc.declare_dram_parameter("wsa", [H, H], BF16, isOutput=False)
    wst = nc.declare_dram_parameter("wst", [H, A], BF16, isOutput=False)
    wha = nc.declare_dram_parameter("wha", [H, H], BF16, isOutput=False)
    wht = nc.declare_dram_parameter("wht", [H, A], BF16, isOutput=False)
    wva = nc.declare_dram_parameter("wva", [H, A], BF16, isOutput=False)
    wctx = nc.declare_dram_parameter("wctx", [H, H], BF16, isOutput=False)
    walpha = nc.declare_dram_parameter("walpha", [128, KA], BF16, isOutput=False)
    biases = nc.declare_dram_parameter("biases", [128, 32], F32, isOutput=False)

    out_lt = nc.declare_dram_parameter("out_lt", [H, NB], F32, isOutput=True)
    attw_o = nc.declare_dram_parameter("attw", [NB, P + 1], F32, isOutput=True)

    with tile.TileContext(nc) as tc:
        with (
            tc.tile_pool(name="const", bufs=1) as cp,
            tc.tile_pool(name="dram", bufs=1, space="DRAM") as dp,
            tc.tile_pool(name="xtp", bufs=3) as xtp,
            tc.tile_pool(name="xnp", bufs=2) as xnp,
            tc.tile_pool(name="tvp", bufs=2) as tvp,
        ):
            # ---- constant loads (sync HWDGE queue) ------------------------
            # order matters: small tensors, then weights in first-use order.
            walpha_sb = cp.tile([128, KA], BF16)
            nc.sync.dma_start(walpha_sb[:], walpha.ap())
            biases_sb = cp.tile([128, 32], F32)
            nc.sync.dma_start(biases_sb[:], biases.ap())
            dot_sb = cp.tile([128, KH, NB], BF16)
            nc.sync.dma_start(dot_sb[:], dot_t.ap().rearrange("(k ki) b -> ki k b", ki=128))
            st_sb = cp.tile([128, KH, NB], BF16)
            nc.sync.dma_start(st_sb[:], st_t.ap().rearrange("(k ki) b -> ki k b", ki=128))

            def load_w(name, ap, cols, eng):
                t = cp.tile([128, KH, cols], BF16, name=name)
                eng.dma_start(t[:], ap.rearrange("(k ki) n -> ki k n", ki=128))
                return t

            # split weight streams across the two HWDGE queues in first-use
            # order so phase B / phase C never wait on a single queue.
            wsa_sb = load_w("wsa_sb", wsa.ap(), H, nc.sync)
            wht_sb = load_w("wht_sb", wht.ap(), A, nc.scalar)
            wst_sb = load_w("wst_sb", wst.ap(), A, nc.scalar)
            wva_sb = load_w("wva_sb", wva.ap(), A, nc.scalar)
            wha_sb = load_w("wha_sb", wha.ap(), H, nc.sync)
            wctx_sb = load_w("wctx_sb", wctx.ap(), H, nc.sync)

            ident_bf = cp.tile([128, 128], BF16)
            nc.gpsimd.memset(ident_bf[:], 0.0)
            nc.gpsimd.affine_select(
                out=ident_bf[:], in_=ident_bf[:],
                compare_op=ALU.not_equal, fill=1.0, base=0,
                pattern=[[-1, 128]], channel_multiplier=1,
            )
            ident_f = cp.tile([32, 32], F32)
            nc.gpsimd.memset(ident_f[:], 0.0)
            nc.gpsimd.affine_select(
                out=ident_f[:], in_=ident_f[:],
                compare_op=ALU.not_equal, fill=1.0, base=0,
                pattern=[[-1, 32]], channel_multiplier=1,
            )
            # block-diagonal mask: mask01[q, c, b] = 1 where row 128c+q
            # belongs to batch b (196b <= 128c+q <= 196b+195), else 0.
            mask01 = cp.tile([128, RC, NB], BF16)
            nc.gpsimd.memset(mask01[:], 1.0)
            nc.gpsimd.affine_select(
                out=mask01[:], in_=mask01[:],
                compare_op=ALU.is_ge, fill=0.0, base=0,
                pattern=[[128, RC], [-P, NB]], channel_multiplier=1)
            nc.gpsimd.affine_select(
                out=mask01[:], in_=mask01[:],
                compare_op=ALU.is_ge, fill=0.0, base=P - 1,
                pattern=[[-128, RC], [P, NB]], channel_multiplier=-1)

            # persistent small tensors
            saT = cp.tile([128, KH, NB], BF16)
            haT = cp.tile([128, KH, NB], BF16)
            hbaV = cp.tile([128, KA, NB], F32)   # h_att + b_h_att + b_v_att
            hbaS = cp.tile([128, KA, NB], F32)   # h_att + b_h_att + b_sen_att
            sen_sum = cp.tile([128, KA, NB], F32)
            sen_col = cp.tile([128, KA, NB], BF16)
            alpha_s_col = cp.tile([NB, 1], F32)
            alpha_sb = cp.tile([1, R], F32)
            expr_sb = cp.tile([1, R], F32)
            alpha_bp = cp.tile([NB, P + 1], F32)
            expw = cp.tile([NB, P + 1], F32)
            rinv = cp.tile([NB, 1], F32)
            rinv_u = cp.tile([NB, 1], F32)
            exp_negm = cp.tile([NB, 1], F32)
            exp_sen_u = cp.tile([NB, 1], F32)
            attw_sb = cp.tile([NB, P + 1], F32)
            w_r = cp.tile([128, 64], BF16)
            wblk = cp.tile([128, RC, NB], BF16)
            sa_nat = cp.tile([NB, H], BF16)
            ctx_nat = cp.tile([NB, H], F32)
            sen_term = cp.tile([NB, H], F32)
            ctxha = cp.tile([128, KH, NB], BF16)
            out_sb = cp.tile([128, KH, NB], F32)
            alpha_d = dp.tile([R], F32)
            w_d = dp.tile([64 * 128], BF16)

            # ---- phase B: small paths -------------------------------------
            with tc.tile_pool(name="psB", bufs=1, space="PSUM") as psB:
                for ko in range(KH):
                    ps = psB.tile([128, NB], F32, tag="pb", bufs=2)
                    for k in range(KH):
                        nc.tensor.matmul(
                            ps[:], wsa_sb[:, k, ko * 128:(ko + 1) * 128],
                            st_sb[:, k, :], start=(k == 0), stop=(k == KH - 1))
                    nc.scalar.activation(saT[:, ko, :], ps[:], AF.Relu,
                                         bias=biases_sb[:, ko:ko + 1])
                for ko in range(KH):
                    ps = psB.tile([128, NB], F32, tag="pb", bufs=2)
                    for k in range(KH):
                        nc.tensor.matmul(
                            ps[:], wha_sb[:, k, ko * 128:(ko + 1) * 128],
                            dot_sb[:, k, :], start=(k == 0), stop=(k == KH - 1))
                    nc.scalar.activation(haT[:, ko, :], ps[:], AF.Tanh,
                                         bias=biases_sb[:, 8 + ko:9 + ko])
                for ao in range(KA):
                    ps = psB.tile([128, NB], F32, tag="pb", bufs=2)
                    for k in range(KH):
                        nc.tensor.matmul(
                            ps[:], wht_sb[:, k, ao * 128:(ao + 1) * 128],
                            haT[:, k, :], start=(k == 0), stop=(k == KH - 1))
                    nc.scalar.activation(hbaV[:, ao, :], ps[:], AF.Identity,
                                         bias=biases_sb[:, 28 + ao:29 + ao])
                    nc.scalar.activation(hbaS[:, ao, :], ps[:], AF.Identity,
                                         bias=biases_sb[:, 24 + ao:25 + ao])
                for ao in range(KA):
                    ps = psB.tile([128, NB], F32, tag="pb", bufs=2)
                    for k in range(KH):
                        nc.tensor.matmul(
                            ps[:], wst_sb[:, k, ao * 128:(ao + 1) * 128],
                            saT[:, k, :], start=(k == 0), stop=(k == KH - 1))
                    nc.vector.tensor_tensor(sen_sum[:, ao, :], ps[:],
                                            hbaS[:, ao, :], op=ALU.add)
                    nc.scalar.activation(sen_col[:, ao, :], sen_sum[:, ao, :],
                                         AF.Tanh)
                ps_asen = psB.tile([NB, 1], F32, tag="pasen", bufs=1)
                for ao in range(KA):
                    nc.tensor.matmul(ps_asen[:], sen_col[:, ao, :],
                                     walpha_sb[:, ao:ao + 1],
                                     start=(ao == 0), stop=(ao == KA - 1))
                nc.scalar.copy(alpha_s_col[:], ps_asen[:])
                nc.vector.tensor_copy(alpha_bp[:, P:P + 1], alpha_s_col[:])

            # ---- phase C: big matmul + alpha ------------------------------
            with tc.tile_pool(name="psC", bufs=1, space="PSUM") as psC:
                xt_r = xt.ap().rearrange("(k ki) r -> ki k r", ki=128)
                ad_r = alpha_d.rearrange("(b p) -> b p", b=NB)
                for g in range(NG):
                    xt_g = xtp.tile([128, KH, GR], BF16, tag="xtg")
                    nc.scalar.dma_start(xt_g[:], xt_r[:, :, g * GR:(g + 1) * GR])
                    tvts = []
                    for ao in range(KA):
                        ps = psC.tile([128, GR], F32, tag="pv", bufs=5)
                        for k in range(KH):
                            nc.tensor.matmul(
                                ps[:], wva_sb[:, k, ao * 128:(ao + 1) * 128],
                                xt_g[:, k, :], start=(k == 0), stop=(k == KH - 1))
                        tv = tvp.tile([128, GR], BF16, tag=f"tv{ao}")
                        nc.vector.tensor_tensor(
                            tv.rearrange("p (b q) -> p b q", q=P),
                            ps.rearrange("p (b q) -> p b q", q=P),
                            hbaV[:, ao, g * GB:(g + 1) * GB, None].to_broadcast(
                                (128, GB, P)),
                            op=ALU.add)
                        tvt = tvp.tile([128, GR], BF16, tag=f"tvt{ao}")
                        nc.scalar.activation(tvt[:], tv[:], AF.Tanh)
                        tvts.append(tvt)
                    ps_a = psC.tile([1, GR], F32, tag="pa", bufs=2)
                    for ao in range(KA):
                        nc.tensor.matmul(ps_a[:], walpha_sb[:, ao:ao + 1],
                                         tvts[ao][:],
                                         start=(ao == 0), stop=(ao == KA - 1))
                    nc.scalar.copy(alpha_sb[:, g * GR:(g + 1) * GR], ps_a[:])
                    # unshifted exp(alpha) in r-major order, straight to DRAM
                    # as bf16 (gpsimd casts); feeds the Wblk xbar transpose.
                    nc.scalar.activation(expr_sb[:, g * GR:(g + 1) * GR],
                                         ps_a[:], AF.Exp)
                    nc.gpsimd.dma_start(w_d[g * GR:(g + 1) * GR],
                                        expr_sb[:, g * GR:(g + 1) * GR])
                    # bounce this group's alpha into [b, p] layout (gpsimd
                    # SWDGE; overlaps with later groups' compute)
                    nc.gpsimd.dma_start(alpha_d[g * GR:(g + 1) * GR],
                                        alpha_sb[:, g * GR:(g + 1) * GR])
                    nc.gpsimd.dma_start(alpha_bp[g * GB:(g + 1) * GB, :P],
                                        ad_r[g * GB:(g + 1) * GB, :])

            # ---- phase D: transposes for sentinel + softmax ---------------
            with tc.tile_pool(name="psD", bufs=1, space="PSUM") as psD:
                # sa_nat = transpose(saT)  (keeps PE warm during softmax)
                for ko in range(KH):
                    pt = psD.tile([NB, 128], BF16, tag="ptr_sa", bufs=2)
                    nc.tensor.transpose(pt[:], saT[:, ko, :], ident_bf[:])
                    nc.vector.tensor_copy(sa_nat[:, ko * 128:(ko + 1) * 128], pt[:])

                # Wblk critical path: load the bf16 unshifted exp weights as
                # [64, 128], PE-transpose to r-major [128, 64], then one DVE
                # multiply with the precomputed block-diagonal mask.
                w_sb = cp.tile([64, 128], BF16)
                nc.scalar.dma_start(w_sb[:], w_d.rearrange("(c q) -> c q", q=128))
                pwr = psD.tile([128, 64], BF16, tag="pwr", bufs=1)
                nc.tensor.transpose(pwr[:], w_sb[:], ident_bf[:64, :64])
                nc.vector.tensor_copy(w_r[:], pwr[:])
                nc.vector.tensor_tensor(
                    wblk[:], mask01[:],
                    w_r[:, :RC, None].to_broadcast((128, RC, NB)), op=ALU.mult)

                # softmax (shifted) for the attention-weights output
                negm = cp.tile([NB, 1], F32)
                nc.vector.tensor_reduce(negm[:], alpha_bp[:], axis=AX.X,
                                        op=ALU.max, negate=True)
                nc.scalar.activation(expw[:], alpha_bp[:], AF.Exp, bias=negm[:])
                ssum = cp.tile([NB, 1], F32)
                nc.vector.tensor_reduce(ssum[:], expw[:], axis=AX.X, op=ALU.add)
                nc.vector.reciprocal(rinv[:], ssum[:])
                nc.vector.tensor_scalar_mul(attw_sb[:], expw[:], rinv[:])
                nc.gpsimd.dma_start(attw_o.ap(), attw_sb[:])
                # normalizers for the UNSHIFTED exp used in the context path:
                # 1/sum_u = rinv * exp(-max);  sentinel exp_u = exp(alpha_sen)
                nc.scalar.activation(exp_negm[:], negm[:], AF.Exp)
                nc.vector.tensor_mul(rinv_u[:], rinv[:], exp_negm[:])
                nc.scalar.activation(exp_sen_u[:], alpha_s_col[:], AF.Exp)

            # ---- phase E: context + final ---------------------------------
            with tc.tile_pool(name="psE", bufs=1, space="PSUM") as psE:
                pctx0 = psE.tile([NB, 512], F32, tag="pctx", bufs=2)
                pctx1 = psE.tile([NB, 512], F32, tag="pctx", bufs=2)
                xn_r = xn.ap().rearrange("(t j q) h -> t q j h", q=128, j=XNC)
                for t in range(RC // XNC):
                    xn_t = xnp.tile([128, XNC, H], BF16, tag="xnt")
                    nc.sync.dma_start(xn_t[:], xn_r[t])
                    for j in range(XNC):
                        c = t * XNC + j
                        nc.tensor.matmul(pctx0[:], wblk[:, c, :], xn_t[:, j, :512],
                                         start=(c == 0), stop=(c == RC - 1))
                        nc.tensor.matmul(pctx1[:], wblk[:, c, :], xn_t[:, j, 512:],
                                         start=(c == 0), stop=(c == RC - 1))
                # sentinel term, then fold in the unshifted-1/sum normalization
                nc.vector.tensor_scalar_mul(sen_term[:], sa_nat[:],
                                            exp_sen_u[:])
                nc.vector.tensor_tensor(ctx_nat[:, :512], pctx0[:],
                                        sen_term[:, :512], op=ALU.add)
                nc.vector.tensor_tensor(ctx_nat[:, 512:], pctx1[:],
                                        sen_term[:, 512:], op=ALU.add)
                nc.vector.tensor_scalar_mul(ctx_nat[:], ctx_nat[:], rinv_u[:])
                # transpose context to [h, b] and add haT
                for ko in range(KH):
                    pt = psE.tile([128, NB], F32, tag="ptr_cx", bufs=2)
                    nc.tensor.transpose(pt[:], ctx_nat[:, ko * 128:(ko + 1) * 128],
                                        ident_f[:])
                    nc.vector.tensor_tensor(ctxha[:, ko, :], pt[:], haT[:, ko, :],
                                            op=ALU.add)
                # out_l^T = tanh(Wctx^T @ ctxha + bctx)
                for ho in range(KH):
                    ps = psE.tile([128, NB], F32, tag="po", bufs=2)
                    for k in range(KH):
                        nc.tensor.matmul(
                            ps[:], wctx_sb[:, k, ho * 128:(ho + 1) * 128],
                            ctxha[:, k, :], start=(k == 0), stop=(k == KH - 1))
                    nc.scalar.activation(out_sb[:, ho, :], ps[:], AF.Tanh,
                                         bias=biases_sb[:, 16 + ho:17 + ho])
                nc.gpsimd.dma_start(
                    out_lt.ap().rearrange("(k ki) b -> ki k b", ki=128), out_sb[:])

    nc.compile()
    return nc


_NC_CACHE = None


def _get_nc():
    global _NC_CACHE
    if _NC_CACHE is None:
        _NC_CACHE = build_nc()
    return _NC_CACHE


def _prep_in_maps(inputs):
    f32 = np.float32
    si = np.asarray(inputs["spatial_image"], f32)
    dec = np.asarray(inputs["decoder_out"], f32)
    st = np.asarray(inputs["st"], f32)

    def bf(x):
        return np.ascontiguousarray(x).astype(_BF)

    shared = {
        "wsa": bf(inputs["W_sen_aff"]),
        "wst": bf(inputs["W_sen_att"]),
        "wha": bf(inputs["W_h_aff"]),
        "wht": bf(inputs["W_h_att"]),
        "wva": bf(inputs["W_v_att"]),
        "wctx": bf(inputs["W_ctx"]),
        "walpha": bf(np.asarray(inputs["W_alpha"], f32)[:, 0].reshape(KA, 128).T),
    }
    biases = np.zeros((128, 32), f32)
    biases[:, 0:8] = np.asarray(inputs["b_sen_aff"], f32).reshape(8, 128).T
    biases[:, 8:16] = np.asarray(inputs["b_h_aff"], f32).reshape(8, 128).T
    biases[:, 16:24] = np.asarray(inputs["b_ctx"], f32).reshape(8, 128).T
    bst_bht = (np.asarray(inputs["b_sen_att"], f32)
               + np.asarray(inputs["b_h_att"], f32))
    bvt_bht = (np.asarray(inputs["b_v_att"], f32)
               + np.asarray(inputs["b_h_att"], f32))
    biases[:, 24:28] = bst_bht.reshape(KA, 128).T
    biases[:, 28:32] = bvt_bht.reshape(KA, 128).T
    shared["biases"] = biases

    in_maps = []
    for c in range(NCORES):
        sl = slice(c * NB, (c + 1) * NB)
        xs = si[sl].reshape(R, H)
        m = dict(shared)
        m["xt"] = bf(xs.T)
        m["xn"] = xs.astype(_BF)
        m["dot_t"] = bf(dec[sl].T)
        m["st_t"] = bf(st[sl].T)
        in_maps.append(m)
    return in_maps


def _run(inputs, **kwargs):
    nc = _get_nc()
    in_maps = _prep_in_maps(inputs)
    res = run_bass_kernel_spmd(nc, in_maps, core_ids=list(range(NCORES)), **kwargs)
    out_l = np.empty((B, H), np.float32)
    attw = np.empty((B, P + 1), np.float32)
    for c in range(NCORES):
        sl = slice(c * NB, (c + 1) * NB)
        out_l[sl] = np.asarray(res.results[c]["out_lt"]).T
        attw[sl] = np.asarray(res.results[c]["attw"])
    beta = attw[:, P:P + 1].copy()
    return (out_l, attw, beta), res


def kernel(**inputs):
    outs, _ = _run(inputs)
    return outs


# revision 22
# speedup vs baseline: 1.0579x; 1.0579x over previous
"""Trainium2 Bass kernel for nn_AdaptiveAttention (8-core data parallel).

Sharding strategy (host side, inside kernel()):
  - batch dim (256) sharded over 8 cores -> 32 batches/core.
  - all weights replicated (passed bf16; compute is bf16 on the PE).
  - spatial_image shard is passed in BOTH layouts as bf16:
      xt = X^T [1024, 6272]  (features on partitions; rhs of the big matmul)
      xn = X   [6272, 1024]  (rows on partitions; rhs of the context matmul)
    Two bf16 copies == same HBM bytes as one f32 copy.
  - small per-batch tensors / biases are passed pre-transposed / pre-tiled so
    every on-chip bias is per-partition and no on-chip transposes of inputs
    are needed.

On-chip math (per core, "transposed world": features on partitions, batch on
free dim):
  saT  = relu(Wsa^T @ st^T + bsa)            [1024, 32]
  haT  = tanh(Wha^T @ dec^T + bha)           [1024, 32]
  hatT = Wht^T @ haT                          [512, 32]
  senT = tanh(Wst^T @ saT + hatT + bst+bht)   [512, 32]
  alpha_sen[b] = walpha . senT[:, b]          via PE (lhsT=senT chunk)
  V^T  = Wva^T @ X^T (+ hatT + bvt+bht bcast) -> tanh -> alpha = walpha . (.)
  softmax over [32, 197] (b on partitions) -> att_weights (output)
  context = Wblk^T @ X + e_sen * sa           (Wblk = block-diag of UNNORMALIZED
                                               exp weights: mask * exp; the
                                               1/sum normalization is folded
                                               into the context epilogue)
  out_l^T = tanh(Wctx^T @ (context^T + haT) + bctx)   [1024, 32]
"""

import numpy as np
import ml_dtypes

import concourse.bass as bass
import concourse.bacc as bacc
import concourse.tile as tile
import concourse.mybir as mybir
from concourse.bass_utils import run_bass_kernel_spmd

BF16 = mybir.dt.bfloat16
F32 = mybir.dt.float32
AF = mybir.ActivationFunctionType
ALU = mybir.AluOpType
AX = mybir.AxisListType

NCORES = 8
B, P, H, A = 256, 196, 1024, 512
NB = B // NCORES            # 32 batches per core
R = NB * P                  # 6272 rows per core
KH = H // 128               # 8 h-chunks
KA = A // 128               # 4 a-chunks
GB = 2                      # batches per V-group
NG = NB // GB               # 16 groups
GR = GB * P                 # 392 rows per group
RC = R // 128               # 49 row-chunks for context
XNC = 7                     # xn tiles consolidated per DMA

_BF = ml_dtypes.bfloat16


def build_nc():
    nc = bacc.Bacc(None, target_bir_lowering=False, debug=False)

    xt = nc.declare_dram_parameter("xt", [H, R], BF16, isOutput=False)
    xn = nc.declare_dram_parameter("xn", [R, H], BF16, isOutput=False)
    dot_t = nc.declare_dram_parameter("dot_t", [H, NB], BF16, isOutput=False)
    st_t = nc.declare_dram_parameter("st_t", [H, NB], BF16, isOutput=False)
    wsa = nc.declare_dram_parameter("wsa", [H, H], BF16, isOutput=False)
    wst = nc.declare_dram_parameter("wst", [H, A], BF16, isOutput=False)
    wha = nc.declare_dram_parameter("wha", [H, H], BF16, isOutput=False)
    wht = nc.declare_dram_parameter("wht", [H, A], BF16, isOutput=False)
    wva = nc.declare_dram_parameter("wva", [H, A], BF16, isOutput=False)
    wctx = nc.declare_dram_parameter("wctx", [H, H], BF16, isOutput=False)
    walpha = nc.declare_dram_parameter("walpha", [128, KA], BF16, isOutput=False)
    biases = nc.declare_dram_parameter("biases", [128, 32], F32, isOutput=False)

    out_lt = nc.declare_dram_parameter("out_lt", [H, NB], F32, isOutput=True)
    attw_o = nc.declare_dram_parameter("attw", [NB, P + 1], F32, isOutput=True)

    with tile.TileContext(nc) as tc:
        with (
            tc.tile_pool(name="const", bufs=1) as cp,
            tc.tile_pool(name="dram", bufs=1, space="DRAM") as dp,
            tc.tile_pool(name="xtp", bufs=2) as xtp,
            tc.tile_pool(name="xnp", bufs=4) as xnp,
            tc.tile_pool(name="tvp", bufs=2) as tvp,
        ):
            # ---- constant loads (sync HWDGE queue) ------------------------
            # order matters: small tensors, then weights in first-use order.
            walpha_sb = cp.tile([128, KA], BF16)
            nc.sync.dma_start(walpha_sb[:], walpha.ap())
            biases_sb = cp.tile([128, 32], F32)
            nc.sync.dma_start(biases_sb[:], biases.ap())
            dot_sb = cp.tile([128, KH, NB], BF16)
            nc.sync.dma_start(dot_sb[:], dot_t.ap().rearrange("(k ki) b -> ki k b", ki=128))
            st_sb = cp.tile([128, KH, NB], BF16)
            nc.sync.dma_start(st_sb[:], st_t.ap().rearrange("(k ki) b -> ki k b", ki=128))

            def load_w(name, ap, cols, eng):
                t = cp.tile([128, KH, cols], BF16, name=name)
                eng.dma_start(t[:], ap.rearrange("(k ki) n -> ki k n", ki=128))
                return t

            # split weight streams across the two HWDGE queues in first-use
            # order so phase B / phase C never wait on a single queue.
            wsa_sb = load_w("wsa_sb", wsa.ap(), H, nc.sync)
            wht_sb = load_w("wht_sb", wht.ap(), A, nc.scalar)
            wst_sb = load_w("wst_sb", wst.ap(), A, nc.scalar)
            wva_sb = load_w("wva_sb", wva.ap(), A, nc.scalar)
            wha_sb = load_w("wha_sb", wha.ap(), H, nc.sync)
            wctx_sb = load_w("wctx_sb", wctx.ap(), H, nc.sync)

            ident_bf = cp.tile([128, 128], BF16)
            nc.gpsimd.memset(ident_bf[:], 0.0)
            nc.gpsimd.affine_select(
                out=ident_bf[:], in_=ident_bf[:],
                compare_op=ALU.not_equal, fill=1.0, base=0,
                pattern=[[-1, 128]], channel_multiplier=1,
            )
            ident_f = cp.tile([32, 32], F32)
            nc.gpsimd.memset(ident_f[:], 0.0)
            nc.gpsimd.affine_select(
                out=ident_f[:], in_=ident_f[:],
                compare_op=ALU.not_equal, fill=1.0, base=0,
                pattern=[[-1, 32]], channel_multiplier=1,
            )
            # block-diagonal mask: mask01[q, c, b] = 1 where row 128c+q
            # belongs to batch b (196b <= 128c+q <= 196b+195), else 0.
            mask01 = cp.tile([128, RC, NB], BF16)
            nc.gpsimd.memset(mask01[:], 1.0)
            nc.gpsimd.affine_select(
                out=mask01[:], in_=mask01[:],
                compare_op=ALU.is_ge, fill=0.0, base=0,
                pattern=[[128, RC], [-P, NB]], channel_multiplier=1)
            nc.gpsimd.affine_select(
                out=mask01[:], in_=mask01[:],
                compare_op=ALU.is_ge, fill=0.0, base=P - 1,
                pattern=[[-128, RC], [P, NB]], channel_multiplier=-1)

            # persistent small tensors
            saT = cp.tile([128, KH, NB], BF16)
            haT = cp.tile([128, KH, NB], BF16)
            hbaV = cp.tile([128, KA, NB], F32)   # h_att + b_h_att + b_v_att
            hbaS = cp.tile([128, KA, NB], F32)   # h_att + b_h_att + b_sen_att
            sen_sum = cp.tile([128, KA, NB], F32)
            sen_col = cp.tile([128, KA, NB], BF16)
            expr_sb = cp.tile([1, R], F32)
            expw_bp = cp.tile([NB, P + 1], F32)
            rinv_u = cp.tile([NB, 1], F32)
            exp_sen_u = cp.tile([NB, 1], F32)
            attw_sb = cp.tile([NB, P + 1], F32)
            w_r = cp.tile([128, 64], BF16)
            wblk = cp.tile([128, RC, NB], BF16)
            sa_nat = cp.tile([NB, H], BF16)
            ctx_nat = cp.tile([NB, H], F32)
            sen_term = cp.tile([NB, H], F32)
            ctxha = cp.tile([128, KH, NB], BF16)
            out_sb = cp.tile([128, KH, NB], F32)
            w_d = dp.tile([64 * 128], BF16)

            # ---- phase B: small paths -------------------------------------
            with tc.tile_pool(name="psB", bufs=1, space="PSUM") as psB:
                for ko in range(KH):
                    ps = psB.tile([128, NB], F32, tag="pb", bufs=2)
                    for k in range(KH):
                        nc.tensor.matmul(
                            ps[:], wsa_sb[:, k, ko * 128:(ko + 1) * 128],
                            st_sb[:, k, :], start=(k == 0), stop=(k == KH - 1))
                    nc.scalar.activation(saT[:, ko, :], ps[:], AF.Relu,
                                         bias=biases_sb[:, ko:ko + 1])
                for ko in range(KH):
                    ps = psB.tile([128, NB], F32, tag="pb", bufs=2)
                    for k in range(KH):
                        nc.tensor.matmul(
                            ps[:], wha_sb[:, k, ko * 128:(ko + 1) * 128],
                            dot_sb[:, k, :], start=(k == 0), stop=(k == KH - 1))
                    nc.scalar.activation(haT[:, ko, :], ps[:], AF.Tanh,
                                         bias=biases_sb[:, 8 + ko:9 + ko])
                for ao in range(KA):
                    ps = psB.tile([128, NB], F32, tag="pb", bufs=2)
                    for k in range(KH):
                        nc.tensor.matmul(
                            ps[:], wht_sb[:, k, ao * 128:(ao + 1) * 128],
                            haT[:, k, :], start=(k == 0), stop=(k == KH - 1))
                    nc.scalar.activation(hbaV[:, ao, :], ps[:], AF.Identity,
                                         bias=biases_sb[:, 28 + ao:29 + ao])
                    nc.scalar.activation(hbaS[:, ao, :], ps[:], AF.Identity,
                                         bias=biases_sb[:, 24 + ao:25 + ao])
                for ao in range(KA):
                    ps = psB.tile([128, NB], F32, tag="pb", bufs=2)
                    for k in range(KH):
                        nc.tensor.matmul(
                            ps[:], wst_sb[:, k, ao * 128:(ao + 1) * 128],
                            saT[:, k, :], start=(k == 0), stop=(k == KH - 1))
                    nc.vector.tensor_tensor(sen_sum[:, ao, :], ps[:],
                                            hbaS[:, ao, :], op=ALU.add)
                    nc.scalar.activation(sen_col[:, ao, :], sen_sum[:, ao, :],
                                         AF.Tanh)
                ps_asen = psB.tile([NB, 1], F32, tag="pasen", bufs=1)
                for ao in range(KA):
                    nc.tensor.matmul(ps_asen[:], sen_col[:, ao, :],
                                     walpha_sb[:, ao:ao + 1],
                                     start=(ao == 0), stop=(ao == KA - 1))
                nc.scalar.activation(exp_sen_u[:], ps_asen[:], AF.Exp)
                nc.vector.tensor_copy(expw_bp[:, P:P + 1], exp_sen_u[:])

            # ---- phase C: big matmul + alpha ------------------------------
            with tc.tile_pool(name="psC", bufs=1, space="PSUM") as psC:
                xt_r = xt.ap().rearrange("(k ki) r -> ki k r", ki=128)
                wd_r = w_d[:R].rearrange("(b p) -> b p", b=NB)
                for g in range(NG):
                    xt_g = xtp.tile([128, KH, GR], BF16, tag="xtg")
                    nc.scalar.dma_start(xt_g[:], xt_r[:, :, g * GR:(g + 1) * GR])
                    tvts = []
                    for ao in range(KA):
                        ps = psC.tile([128, GR], F32, tag="pv", bufs=5)
                        for k in range(KH):
                            nc.tensor.matmul(
                                ps[:], wva_sb[:, k, ao * 128:(ao + 1) * 128],
                                xt_g[:, k, :], start=(k == 0), stop=(k == KH - 1))
                        tv = tvp.tile([128, GR], BF16, tag=f"tv{ao}")
                        nc.vector.tensor_tensor(
                            tv.rearrange("p (b q) -> p b q", q=P),
                            ps.rearrange("p (b q) -> p b q", q=P),
                            hbaV[:, ao, g * GB:(g + 1) * GB, None].to_broadcast(
                                (128, GB, P)),
                            op=ALU.add)
                        tvt = tvp.tile([128, GR], BF16, tag=f"tvt{ao}")
                        nc.scalar.activation(tvt[:], tv[:], AF.Tanh)
                        tvts.append(tvt)
                    ps_a = psC.tile([1, GR], F32, tag="pa", bufs=2)
                    for ao in range(KA):
                        nc.tensor.matmul(ps_a[:], walpha_sb[:, ao:ao + 1],
                                         tvts[ao][:],
                                         start=(ao == 0), stop=(ao == KA - 1))
                    # unshifted exp(alpha) in r-major order, straight to DRAM
                    # as bf16 (gpsimd casts); feeds both the Wblk transpose
                    # and the [b, p]-layout attention-weight output.
                    nc.scalar.activation(expr_sb[:, g * GR:(g + 1) * GR],
                                         ps_a[:], AF.Exp)
                    nc.gpsimd.dma_start(w_d[g * GR:(g + 1) * GR],
                                        expr_sb[:, g * GR:(g + 1) * GR])
                    nc.gpsimd.dma_start(expw_bp[g * GB:(g + 1) * GB, :P],
                                        wd_r[g * GB:(g + 1) * GB, :])

            # ---- phase D: transposes for sentinel + softmax ---------------
            with tc.tile_pool(name="psD", bufs=1, space="PSUM") as psD:
                # sa_nat = transpose(saT)  (keeps PE warm during softmax)
                for ko in range(KH):
                    pt = psD.tile([NB, 128], BF16, tag="ptr_sa", bufs=2)
                    nc.tensor.transpose(pt[:], saT[:, ko, :], ident_bf[:])
                    nc.vector.tensor_copy(sa_nat[:, ko * 128:(ko + 1) * 128], pt[:])

                # Wblk critical path: load the bf16 unshifted exp weights as
                # [64, 128], PE-transpose to r-major [128, 64], then one DVE
                # multiply with the precomputed block-diagonal mask.
                w_sb = cp.tile([64, 128], BF16)
                nc.scalar.dma_start(w_sb[:], w_d.rearrange("(c q) -> c q", q=128))
                pwr = psD.tile([128, 64], BF16, tag="pwr", bufs=1)
                nc.tensor.transpose(pwr[:], w_sb[:], ident_bf[:64, :64])
                nc.vector.tensor_copy(w_r[:], pwr[:])
                nc.vector.tensor_tensor(
                    wblk[:], mask01[:],
                    w_r[:, :RC, None].to_broadcast((128, RC, NB)), op=ALU.mult)

                # normalize the unshifted exp for the attention-weight output
                ssum = cp.tile([NB, 1], F32)
                nc.vector.tensor_reduce(ssum[:], expw_bp[:], axis=AX.X,
                                        op=ALU.add)
                nc.vector.reciprocal(rinv_u[:], ssum[:])
                nc.vector.tensor_scalar_mul(attw_sb[:], expw_bp[:], rinv_u[:])
                nc.gpsimd.dma_start(attw_o.ap(), attw_sb[:])

            # ---- phase E: context + final ---------------------------------
            with tc.tile_pool(name="psE", bufs=1, space="PSUM") as psE:
                pctx0 = psE.tile([NB, 512], F32, tag="pctx", bufs=2)
                pctx1 = psE.tile([NB, 512], F32, tag="pctx", bufs=2)
                xn_r = xn.ap().rearrange("(t j q) h -> t q j h", q=128, j=XNC)
                for t in range(RC // XNC):
                    xn_t = xnp.tile([128, XNC, H], BF16, tag="xnt")
                    nc.sync.dma_start(xn_t[:], xn_r[t])
                    for j in range(XNC):
                        c = t * XNC + j
                        nc.tensor.matmul(pctx0[:], wblk[:, c, :], xn_t[:, j, :512],
                                         start=(c == 0), stop=(c == RC - 1))
                        nc.tensor.matmul(pctx1[:], wblk[:, c, :], xn_t[:, j, 512:],
                                         start=(c == 0), stop=(c == RC - 1))
                # sentinel term, then fold in the unshifted-1/sum normalization
                nc.vector.tensor_scalar_mul(sen_term[:], sa_nat[:],
                                            exp_sen_u[:])
                nc.vector.tensor_tensor(ctx_nat[:, :512], pctx0[:],
                                        sen_term[:, :512], op=ALU.add)
                nc.vector.tensor_tensor(ctx_nat[:, 512:], pctx1[:],
                                        sen_term[:, 512:], op=ALU.add)
                nc.vector.tensor_scalar_mul(ctx_nat[:], ctx_nat[:], rinv_u[:])
                # transpose context to [h, b] and add haT
                for ko in range(KH):
                    pt = psE.tile([128, NB], F32, tag="ptr_cx", bufs=2)
                    nc.tensor.transpose(pt[:], ctx_nat[:, ko * 128:(ko + 1) * 128],
                                        ident_f[:])
                    nc.vector.tensor_tensor(ctxha[:, ko, :], pt[:], haT[:, ko, :],
                                            op=ALU.add)
                # out_l^T = tanh(Wctx^T @ ctxha + bctx)
                for ho in range(KH):
                    ps = psE.tile([128, NB], F32, tag="po", bufs=2)
                    for k in range(KH):
                        nc.tensor.matmul(
                            ps[:], wctx_sb[:, k, ho * 128:(ho + 1) * 128],
                            ctxha[:, k, :], start=(k == 0), stop=(k == KH - 1))
                    nc.scalar.activation(out_sb[:, ho, :], ps[:], AF.Tanh,
                                         bias=biases_sb[:, 16 + ho:17 + ho])
                nc.gpsimd.dma_start(
                    out_lt.ap().rearrange("(k ki) b -> ki k b", ki=128), out_sb[:])

    nc.compile()
    return nc


_NC_CACHE = None


def _get_nc():
    global _NC_CACHE
    if _NC_CACHE is None:
        _NC_CACHE = build_nc()
    return _NC_CACHE


def _prep_in_maps(inputs):
    f32 = np.float32
    si = np.asarray(inputs["spatial_image"], f32)
    dec = np.asarray(inputs["decoder_out"], f32)
    st = np.asarray(inputs["st"], f32)

    def bf(x):
        return np.ascontiguousarray(x).astype(_BF)

    shared = {
        "wsa": bf(inputs["W_sen_aff"]),
        "wst": bf(inputs["W_sen_att"]),
        "wha": bf(inputs["W_h_aff"]),
        "wht": bf(inputs["W_h_att"]),
        "wva": bf(inputs["W_v_att"]),
        "wctx": bf(inputs["W_ctx"]),
        "walpha": bf(np.asarray(inputs["W_alpha"], f32)[:, 0].reshape(KA, 128).T),
    }
    biases = np.zeros((128, 32), f32)
    biases[:, 0:8] = np.asarray(inputs["b_sen_aff"], f32).reshape(8, 128).T
    biases[:, 8:16] = np.asarray(inputs["b_h_aff"], f32).reshape(8, 128).T
    biases[:, 16:24] = np.asarray(inputs["b_ctx"], f32).reshape(8, 128).T
    bst_bht = (np.asarray(inputs["b_sen_att"], f32)
               + np.asarray(inputs["b_h_att"], f32))
    bvt_bht = (np.asarray(inputs["b_v_att"], f32)
               + np.asarray(inputs["b_h_att"], f32))
    biases[:, 24:28] = bst_bht.reshape(KA, 128).T
    biases[:, 28:32] = bvt_bht.reshape(KA, 128).T
    shared["biases"] = biases

    in_maps = []
    for c in range(NCORES):
        sl = slice(c * NB, (c + 1) * NB)
        xs = si[sl].reshape(R, H)
        m = dict(shared)
        m["xt"] = bf(xs.T)
        m["xn"] = xs.astype(_BF)
        m["dot_t"] = bf(dec[sl].T)
        m["st_t"] = bf(st[sl].T)
        in_maps.append(m)
    return in_maps


def _run(inputs, **kwargs):
    nc = _get_nc()
    in_maps = _prep_in_maps(inputs)
    res = run_bass_kernel_spmd(nc, in_maps, core_ids=list(range(NCORES)), **kwargs)
    out_l = np.empty((B, H), np.float32)
    attw = np.empty((B, P + 1), np.float32)
    for c in range(NCORES):
        sl = slice(c * NB, (c + 1) * NB)
        out_l[sl] = np.asarray(res.results[c]["out_lt"]).T
        attw[sl] = np.asarray(res.results[c]["attw"])
    beta = attw[:, P:P + 1].copy()
    return (out_l, attw, beta), res


def kernel(**inputs):
    outs, _ = _run(inputs)
    return outs
